# revision 35
# baseline (speedup 1.0000x reference)
"""Trainium2 Bass kernel for nn_Block_5875515261621 (dense transformer block).

B=2, T=4096, C=512, H=8 heads (hd=64): causal attention + tanh-gelu MLP,
LayerNorms with residuals.

Strategy (8 NeuronCores, two SPMD launches):
  Launch A (attention): core c -> batch b=c//4, head-pair hp=c%4.
    LN1 (bn_stats; rstd via ACT Sqrt + DVE reciprocal -- no Ln, so no
    activation-table thrashing), q/k/v for its 2 heads, causal attention in
    S^T layout (softmax denominators via a ones-column appended to V).
    Softmax exp is split across engines: the scalar engine (ACT) computes
    exact exp for a tunable fraction of key blocks; the vector engine (DVE)
    computes the rest with a one-instruction Schraudolph bit-trick exp
    (y = bitcast_i32(round(s*2^23/ln2 + B))), which for diagonal blocks also
    fuses the causal mask (additive mask baked into the Schraudolph offset).
    End-to-end output error from the approx exp is ~5e-3 (gate is 2e-2).
  Host: concatenates per-core y^T into per-batch y^T [512, 4096]; also folds
    b_attn_proj into x (x' = x + b_ap) so the MLP launch needs no bias there.
  Launch B (proj+MLP): core c -> 1024 tokens. x2 = x' + attn_proj(y);
    LN2 (stats via PE ones-reduction; rstd via Sqrt+reciprocal, batched);
    MLP with fused Gelu_apprx_tanh; residual fused with bias via one
    scalar_tensor_tensor; transpose back to token-major.

All matmuls run in float32r (full PE rate) with fp32 PSUM accumulation.
PE transposes are packed 4-to-a-PSUM-bank so evacuation is one wide copy.
LN gains/biases are folded into adjacent weights on the host (exact).
Compiled executables are cached at module level.
"""
import sys

sys.path.insert(0, "/opt/trn_rl_repo")

import numpy as np

import concourse.bacc as bacc
import concourse.tile as tile
from concourse import mybir
from concourse.masks import make_identity

F32 = mybir.dt.float32
F32R = mybir.dt.float32r
BF16 = mybir.dt.bfloat16
I16 = mybir.dt.int16
AF = mybir.ActivationFunctionType
ALU = mybir.AluOpType

T = 4096
C = 512
NT = T // 128
QB = 512
NQB = T // QB
EPS = 1e-5
SCALE = 1.0 / float(np.sqrt(np.float32(C)))
N_CORES = 8

# Schraudolph exp in bf16: exp(s*SCALE) ~= bitcast_bf16(i16(A*s + B)).
# (bf16 keeps the fp32 exponent layout, so the classic bit-trick works with
# 2^7 in place of 2^23; probabilities only need ~2 digits here.)
SCH_A = float(2**7 / np.log(2) * SCALE)        # multiplier on raw scores
SCH_B = float(127 * 2**7 - 0.045 * 2**7)       # offset (0.045 bias corr.)
SCH_MASKED = float(SCH_B - 12000.0)            # masked lanes -> ~2^-96

# Fraction of softmax exp computed exactly on ACT (rest on DVE Schraudolph).
FRAC_ACT = 1.0
RAW_NEG = -1500.0   # additive causal mask in raw-score units (exp -> ~2^-95)


# ---------------------------------------------------------------------------
# Bass programs
# ---------------------------------------------------------------------------

def _build_attn(repeat=None):
    nc = bacc.Bacc("TRN2", target_bir_lowering=False, debug=False)
    xb_d = nc.dram_tensor("xb", [T, C], F32, kind="ExternalInput")
    wqkv_d = nc.dram_tensor("wqkv", [4, 128, 384], F32, kind="ExternalInput")
    bqkv_d = nc.dram_tensor("bqkv", [3, 128], F32, kind="ExternalInput")
    yT_d = nc.dram_tensor("yT", [128, T], F32, kind="ExternalOutput")

    with tile.TileContext(nc) as tc:
        def body(iv=None):
            with (
                tc.tile_pool(name="big", bufs=1) as big,
                tc.tile_pool(name="stream", bufs=4) as stream,
                tc.tile_pool(name="ptp", bufs=4) as ptp,
                tc.tile_pool(name="small", bufs=4) as small,
            ):
                ident = big.tile([128, 128], F32)
                make_identity(nc, ident[:])
                # Causal mask as a PE-addable constant: the diagonal block's
                # additive mask M[k, q] = RAW_NEG if k > q lands in the score
                # PSUM accumulation group via matmul(maskT, ident), keeping
                # the exp a plain activation.  maskT[q, k] = M[k, q].
                maskF = big.tile([128, 128], F32)
                nc.gpsimd.memset(maskF[:], 0.0)
                nc.gpsimd.affine_select(
                    out=maskF[:], in_=maskF[:],
                    compare_op=ALU.is_ge,
                    fill=RAW_NEG, base=0,
                    pattern=[[-1, 128]], channel_multiplier=1,
                )
                maskT = big.tile([128, 128], F32R)
                nc.vector.tensor_copy(maskT[:], maskF[:])
                # identity zero-extended to 512 cols: the mask matmul spans
                # the whole diagonal-block score slice so the accumulation
                # group gets a single well-formed stop.
                identZF = big.tile([128, 512], F32)
                nc.gpsimd.memset(identZF[:], 0.0)
                make_identity(nc, identZF[:, 0:128])
                identZ = big.tile([128, 512], F32R)
                nc.vector.tensor_copy(identZ[:], identZF[:])

                wq = big.tile([128, 4, 384], F32R)
                nc.scalar.dma_start(
                    wq[:],
                    wqkv_d.ap().rearrange("po pi f -> pi po f").bitcast(F32R),
                )
                bq = big.tile([128, 3], F32)
                nc.scalar.dma_start(bq[:], bqkv_d.ap().rearrange("g p -> p g"))

                xlnT = big.tile([128, 4, T], F32R)
                kT = big.tile([128, T], F32R)
                # Q zero-padded per head: scores then contract over all 128
                # partitions with a shared K stationary (plain matmuls; the
                # zero rows kill the other head's channels). tile_position
                # row-tiling measured ~2x slower than two plain matmuls.
                qTz = big.tile([128, 2, T], F32R)
                nc.gpsimd.memset(qTz[64:128, 0, :].bitcast(F32), 0.0)
                nc.gpsimd.memset(qTz[0:64, 1, :].bitcast(F32), 0.0)
                vT = big.tile([128, T], F32)
                vph = big.tile([128, 2, NT, 65], BF16)
                ones32 = big.tile([128, NT], F32)
                nc.vector.memset(ones32[:], 1.0)
                nc.vector.tensor_copy(vph[:, 0, :, 64:65], ones32[:, :, None])
                nc.vector.tensor_copy(vph[:, 1, :, 64:65], ones32[:, :, None])

                eps_t = big.tile([128, 1], F32)
                nc.vector.memset(eps_t[:], EPS)

                psA, psS, psY = [], [], []

                def p1_tile(it):
                    xt = stream.tile([128, C], F32, tag="xt", name="xt")
                    nc.sync.dma_start(
                        xt[:], xb_d.ap()[it * 128:(it + 1) * 128, :]
                    )
                    st = small.tile([128, 6], F32, tag="st", name="st")
                    mv = small.tile([128, 2], F32, tag="mv", name="mv")
                    nc.vector.bn_stats(st[:], xt[:])
                    nc.vector.bn_aggr(mv[:], st[:])
                    sd = small.tile([128, 1], F32, tag="sd", name="sd")
                    nc.scalar.activation(sd[:], mv[:, 1:2], AF.Sqrt,
                                         bias=eps_t[:])
                    rstd = small.tile([128, 1], F32, tag="rstd", name="rstd")
                    nc.vector.reciprocal(rstd[:], sd[:])
                    xln = stream.tile([128, C], F32, tag="xln", name="xln")
                    nc.vector.tensor_scalar(
                        out=xln[:], in0=xt[:],
                        scalar1=mv[:, 0:1], scalar2=rstd[:],
                        op0=ALU.subtract, op1=ALU.mult,
                    )
                    ptr4 = psA[0].tile([128, 512], F32, tag="tr4", name="tr4")
                    for cs in range(4):
                        nc.tensor.transpose(
                            ptr4[:, cs * 128:(cs + 1) * 128],
                            xln[:, cs * 128:(cs + 1) * 128], ident[:]
                        )
                    src = ptr4[:].rearrange("p (a b) -> p a b", a=4)
                    dst = xlnT[:, :, it * 128:(it + 1) * 128]
                    if it % 2 == 0:
                        nc.vector.tensor_copy(dst, src)
                    else:
                        nc.scalar.copy(dst, src)

                def p2_block(tb):
                    tsl = slice(tb * QB, (tb + 1) * QB)
                    for g in range(3):
                        pq = psA[0].tile([128, QB], F32, tag="qkv", name="qkv")
                        for cs in range(4):
                            nc.tensor.matmul(
                                pq[:],
                                wq[:, cs, g * 128:(g + 1) * 128],
                                xlnT[:, cs, tsl],
                                start=(cs == 0), stop=(cs == 3),
                            )
                        if g == 0:
                            nc.scalar.activation(
                                qTz[0:64, 0, tsl], pq[0:64, :], AF.Identity,
                                bias=bq[0:64, 0:1],
                            )
                            nc.scalar.activation(
                                qTz[64:128, 1, tsl], pq[64:128, :],
                                AF.Identity, bias=bq[64:128, 0:1],
                            )
                        elif g == 1:
                            nc.scalar.activation(
                                kT[:, tsl], pq[:], AF.Identity,
                                bias=bq[:, 1:2],
                            )
                        else:
                            nc.vector.tensor_scalar(
                                out=vT[:, tsl], in0=pq[:],
                                scalar1=bq[:, 2:3], scalar2=None, op0=ALU.add,
                            )

                def p3_tile(it0):
                    ptr4 = psA[0].tile([128, 512], F32, tag="tr4",
                                       name="tr4")
                    for i in range(4):
                        it = it0 + i
                        nc.tensor.transpose(
                            ptr4[:, i * 128:(i + 1) * 128],
                            vT[:, it * 128:(it + 1) * 128], ident[:]
                        )
                    nc.vector.tensor_copy(
                        vph[:, :, it0:it0 + 4, 0:64],
                        ptr4[:].rearrange("p (i h c) -> p h i c", i=4, h=2),
                    )

                LAG = 2
                # Running balance of off-diagonal exp work: ACT vs DVE.
                exp_acc = [1.0, 1.0]  # [act, total]

                def emit_exp_piece(pt, spsum, lo, hi):
                    n = hi - lo
                    if exp_acc[0] < FRAC_ACT * exp_acc[1]:
                        nc.scalar.activation(
                            pt[:, :, lo:hi], spsum[:, :, lo:hi],
                            AF.Exp, scale=SCALE,
                        )
                        exp_acc[0] += n
                    else:
                        nc.vector.tensor_scalar(
                            out=pt[:, :, lo:hi].bitcast(I16),
                            in0=spsum[:, :, lo:hi],
                            scalar1=SCH_A, scalar2=SCH_B,
                            op0=ALU.mult, op1=ALU.add,
                        )
                    exp_acc[1] += n

                def p4_block(qb):
                    nkb = 4 * qb + 4
                    yps = []
                    for h in range(2):
                        ypt = psY[0].tile([65, QB], F32, tag=f"y{h}",
                                          name=f"y{h}")
                        yps.append(ypt)

                    pend = []

                    def emit_av(entry):
                        kb_, off_, pt_ = entry
                        for h in range(2):
                            nc.tensor.matmul(
                                yps[h][:, off_:QB],
                                vph[:, h, kb_, :],
                                pt_[:, h, off_:QB],
                                start=(kb_ == 0), stop=(kb_ == nkb - 1),
                            )

                    for kb in range(nkb):
                        d = kb - 4 * qb
                        off = max(0, d * 128)
                        spsum = psS[0].tile([128, 2, QB], F32, tag="s",
                                            name="s")
                        for h in range(2):
                            nc.tensor.matmul(
                                spsum[:, h, off:QB],
                                kT[:, kb * 128:(kb + 1) * 128],
                                qTz[:, h, qb * QB + off:(qb + 1) * QB],
                                start=True, stop=(d < 0),
                            )
                        if d >= 0:
                            for h in range(2):
                                nc.tensor.matmul(
                                    spsum[:, h, off:QB],
                                    maskT[:],
                                    identZ[:, 0:QB - off],
                                    start=False, stop=True,
                                )
                        pt = ptp.tile([128, 2, QB], BF16, tag="pt", name="pt")
                        emit_exp_piece(pt, spsum, off, QB)
                        pend.append((kb, off, pt))
                        if len(pend) > LAG:
                            emit_av(pend.pop(0))
                    for entry in pend:
                        emit_av(entry)

                    for h in range(2):
                        hsl = slice(h * 64, (h + 1) * 64)
                        recip = small.tile([1, QB], F32, tag="recip",
                                           name="recip")
                        nc.vector.reciprocal(recip[:], yps[h][64:65, :])
                        rb = small.tile([64, QB], F32, tag="rb", name="rb")
                        nc.gpsimd.partition_broadcast(rb[:], recip[:])
                        yst = stream.tile([64, QB], F32, tag="yst",
                                          name="yst")
                        nc.vector.tensor_tensor(
                            out=yst[:], in0=yps[h][0:64, :], in1=rb[:],
                            op=ALU.mult,
                        )
                        nc.sync.dma_start(
                            yT_d.ap()[hsl, qb * QB:(qb + 1) * QB], yst[:]
                        )

                with tc.tile_pool(name="psA", bufs=3,
                                  space="PSUM") as psA_:
                    psA.append(psA_)
                    for it in range(NT):
                        p1_tile(it)
                    for tb in range(NQB):
                        p2_block(tb)
                    for it0 in range(0, NT, 4):
                        p3_tile(it0)
                with (
                    tc.tile_pool(name="psS", bufs=2, space="PSUM") as psS_,
                    tc.tile_pool(name="psY", bufs=2, space="PSUM") as psY_,
                ):
                    psS.append(psS_)
                    psY.append(psY_)
                    for qb in range(NQB):
                        p4_block(qb)

        if repeat is None:
            body()
        else:
            with tc.For_i(0, repeat) as i:
                body(i)

    nc.compile()
    return nc


def _build_mlp(repeat=None):
    TC = 1024            # tokens per core
    NTB = TC // QB       # 2
    nc = bacc.Bacc("TRN2", target_bir_lowering=False, debug=False)
    yTc_d = nc.dram_tensor("yTc", [C, TC], F32, kind="ExternalInput")
    xc_d = nc.dram_tensor("xc", [TC, C], F32, kind="ExternalInput")
    wap_d = nc.dram_tensor("wap", [4, 128, C], F32, kind="ExternalInput")
    wfc_d = nc.dram_tensor("wfc", [4, 128, 4 * C], F32, kind="ExternalInput")
    bfc_d = nc.dram_tensor("bfc", [16, 128], F32, kind="ExternalInput")
    wmp_d = nc.dram_tensor("wmp", [16, 128, C], BF16, kind="ExternalInput")
    bmp_d = nc.dram_tensor("bmp", [4, 128], F32, kind="ExternalInput")
    outc_d = nc.dram_tensor("outc", [TC, C], F32, kind="ExternalOutput")

    with tile.TileContext(nc) as tc:
        def body(iv=None):
            with (
                tc.tile_pool(name="big", bufs=1) as big,
                tc.tile_pool(name="stream", bufs=2) as stream,
                tc.tile_pool(name="hpool", bufs=2) as hpool,
                tc.tile_pool(name="small", bufs=1) as small,
                tc.tile_pool(name="ps", bufs=3, space="PSUM") as ps,
                tc.tile_pool(name="psstat", bufs=1, space="PSUM") as psstat,
                tc.tile_pool(name="pst", bufs=2, space="PSUM") as pst,
            ):
                ident = big.tile([128, 128], F32)
                make_identity(nc, ident[:])

                # weights stream on the ACT HWDGE queue; inputs (yT, x) go
                # first on the SP queue so compute starts immediately.
                wap = big.tile([128, 4, C], F32R)
                wfc = big.tile([128, 4, 4 * C], F32R)
                wmp = big.tile([128, 16, C], BF16)
                nc.scalar.dma_start(
                    wap[:], wap_d.ap().rearrange("po pi f -> pi po f")
                    .bitcast(F32R))
                nc.scalar.dma_start(
                    wfc[:], wfc_d.ap().rearrange("po pi f -> pi po f")
                    .bitcast(F32R))
                nc.scalar.dma_start(
                    wmp[:], wmp_d.ap().rearrange("po pi f -> pi po f"))

                bfc = big.tile([128, 16], F32)
                nc.scalar.dma_start(bfc[:], bfc_d.ap().rearrange("g p -> p g"))
                bmp = big.tile([128, 4], F32)
                nc.scalar.dma_start(bmp[:], bmp_d.ap().rearrange("g p -> p g"))

                yT = big.tile([128, 4, TC], F32R)
                nc.sync.dma_start(
                    yT[:],
                    yTc_d.ap().rearrange("(po pi) t -> pi po t", pi=128)
                    .bitcast(F32R))

                ones_f = big.tile([128, 1], F32)
                nc.vector.memset(ones_f[:], 1.0)
                ones = big.tile([128, 1], F32R)
                nc.vector.tensor_copy(ones[:], ones_f[:])
                eps1 = big.tile([1, 1], F32)
                nc.vector.memset(eps1[:], EPS)

                # x2T starts as x^T (b_attn_proj folded into x on host);
                # attn-proj result is accumulated in below.
                x2T = big.tile([128, 4, TC], F32R)
                for it in range(TC // 128):
                    xt = stream.tile([128, C], F32, tag="xt")
                    nc.sync.dma_start(
                        xt[:], xc_d.ap()[it * 128:(it + 1) * 128, :])
                    ptr4 = pst.tile([128, 512], F32, tag="tr4")
                    for cs in range(4):
                        nc.tensor.transpose(
                            ptr4[:, cs * 128:(cs + 1) * 128],
                            xt[:, cs * 128:(cs + 1) * 128], ident[:]
                        )
                    nc.vector.tensor_copy(
                        x2T[:, :, it * 128:(it + 1) * 128],
                        ptr4[:].rearrange("p (a b) -> p a b", a=4),
                    )

                # attn c_proj for both token blocks, accumulate into x2T
                for tb in range(NTB):
                    tsl = slice(tb * QB, (tb + 1) * QB)
                    for cs in range(4):
                        pq = ps.tile([128, QB], F32, tag="mm")
                        for ks in range(4):
                            nc.tensor.matmul(
                                pq[:],
                                wap[:, ks, cs * 128:(cs + 1) * 128],
                                yT[:, ks, tsl],
                                start=(ks == 0), stop=(ks == 3),
                            )
                        nc.vector.tensor_tensor(
                            out=x2T[:, cs, tsl], in0=pq[:],
                            in1=x2T[:, cs, tsl], op=ALU.add,
                        )

                # LN2 stats via PE ones-reduction; per-tb rstd so tb1's
                # proj/stats matmuls overlap tb0's LN chain.
                mlp_stats = []
                for tb in range(NTB):
                    tsl = slice(tb * QB, (tb + 1) * QB)
                    psum_s = psstat.tile([1, QB], F32, tag="stat_s")
                    psum_q = psstat.tile([1, QB], F32, tag="stat_q")
                    for cs in range(4):
                        nc.tensor.matmul(
                            psum_s[:], ones[:], x2T[:, cs, tsl],
                            start=(cs == 0), stop=(cs == 3),
                        )
                    for cs in range(4):
                        sq = stream.tile([128, QB], F32R, tag="sq")
                        nc.scalar.activation(
                            sq[:], x2T[:, cs, tsl], AF.Square)
                        nc.tensor.matmul(
                            psum_q[:], ones[:], sq[:],
                            start=(cs == 0), stop=(cs == 3),
                        )
                    mu = small.tile([1, QB], F32, tag=f"mu{tb}")
                    nc.vector.tensor_scalar(
                        out=mu[:], in0=psum_s[:],
                        scalar1=1.0 / C, scalar2=None, op0=ALU.mult,
                    )
                    musq = small.tile([1, QB], F32, tag=f"musq{tb}")
                    nc.vector.tensor_tensor(
                        out=musq[:], in0=mu[:], in1=mu[:], op=ALU.mult,
                    )
                    var = small.tile([1, QB], F32, tag=f"var{tb}")
                    nc.vector.tensor_scalar(
                        out=var[:], in0=psum_q[:],
                        scalar1=1.0 / C, scalar2=None, op0=ALU.mult,
                    )
                    nc.vector.tensor_tensor(
                        out=var[:], in0=var[:], in1=musq[:],
                        op=ALU.subtract,
                    )
                    sd = small.tile([1, QB], F32, tag=f"sd{tb}")
                    nc.scalar.activation(sd[:], var[:], AF.Sqrt,
                                         bias=eps1[:])
                    rstd = small.tile([1, QB], F32, tag=f"rstd{tb}")
                    nc.vector.reciprocal(rstd[:], sd[:])
                    mlp_stats.append((mu, rstd))

                for tb in range(NTB):
                    tsl = slice(tb * QB, (tb + 1) * QB)
                    mu, rstd = mlp_stats[tb]
                    mu_b = small.tile([128, QB], F32, tag=f"mu_b{tb}")
                    nc.gpsimd.partition_broadcast(mu_b[:], mu[:])
                    rstd_b = small.tile([128, QB], F32, tag=f"rstd_b{tb}")
                    nc.gpsimd.partition_broadcast(rstd_b[:], rstd[:])

                    xln2 = hpool.tile([128, 4, QB], F32R, tag="xln2")
                    for cs in range(4):
                        nc.vector.tensor_tensor(
                            out=xln2[:, cs, :], in0=x2T[:, cs, tsl],
                            in1=mu_b[:], op=ALU.subtract,
                        )
                        nc.vector.tensor_tensor(
                            out=xln2[:, cs, :], in0=xln2[:, cs, :],
                            in1=rstd_b[:], op=ALU.mult,
                        )

                    # fc + gelu
                    hT = hpool.tile([128, 16, QB], BF16, tag="hT")
                    for fs in range(16):
                        pq = ps.tile([128, QB], F32, tag="mm")
                        for ks in range(4):
                            nc.tensor.matmul(
                                pq[:],
                                wfc[:, ks, fs * 128:(fs + 1) * 128],
                                xln2[:, ks, :],
                                start=(ks == 0), stop=(ks == 3),
                            )
                        nc.scalar.activation(
                            hT[:, fs, :], pq[:], AF.Gelu_apprx_tanh,
                            bias=bfc[:, fs:fs + 1],
                        )

                    # mlp proj; fused bias + residual in one DVE op
                    outT = hpool.tile([128, 4, QB], F32, tag="outT")
                    for cs in range(4):
                        pq = ps.tile([128, QB], F32, tag="mm")
                        for ks in range(16):
                            nc.tensor.matmul(
                                pq[:],
                                wmp[:, ks, cs * 128:(cs + 1) * 128],
                                hT[:, ks, :],
                                start=(ks == 0), stop=(ks == 15),
                            )
                        nc.vector.scalar_tensor_tensor(
                            out=outT[:, cs, :], in0=pq[:],
                            scalar=bmp[:, cs:cs + 1],
                            in1=x2T[:, cs, tsl],
                            op0=ALU.add, op1=ALU.add,
                        )

                    for it in range(QB // 128):
                        ptr4 = pst.tile([128, 512], F32, tag="tr4")
                        for cs in range(4):
                            nc.tensor.transpose(
                                ptr4[:, cs * 128:(cs + 1) * 128],
                                outT[:, cs, it * 128:(it + 1) * 128],
                                ident[:],
                            )
                        ot = stream.tile([128, C], F32, tag="ot")
                        nc.vector.tensor_copy(ot[:], ptr4[:])
                        nc.sync.dma_start(
                            outc_d.ap()[
                                tb * QB + it * 128: tb * QB + (it + 1) * 128,
                                :
                            ],
                            ot[:],
                        )

        if repeat is None:
            body()
        else:
            with tc.For_i(0, repeat) as i:
                body(i)

    nc.compile()
    return nc


# ---------------------------------------------------------------------------
# Memoized SPMD runner (compile once per process)
# ---------------------------------------------------------------------------

class _CompiledSpmd:
    def __init__(self, nc, n_cores):
        import jax
        from jax.sharding import Mesh, PartitionSpec
        from jax.experimental.shard_map import shard_map
        from concourse import bass2jax
        from concourse.bass2jax import _bass_exec_p, partition_id_tensor

        bass2jax.install_neuronx_cc_hook()
        self.jax = jax
        self.n_cores = n_cores
        partition_name = (
            nc.partition_id_tensor.name if nc.partition_id_tensor else None
        )
        in_names, out_names, out_avals, zero_outs = [], [], [], []
        for alloc in nc.m.functions[0].allocations:
            if not isinstance(alloc, mybir.MemoryLocationSet):
                continue
            name = alloc.memorylocations[0].name
            if alloc.kind == "ExternalInput":
                if name != partition_name:
                    in_names.append(name)
            elif alloc.kind == "ExternalOutput":
                shape = tuple(alloc.tensor_shape)
                dtype = mybir.dt.np(alloc.dtype)
                out_names.append(name)
                out_avals.append(jax.core.ShapedArray(shape, dtype))
                zero_outs.append(np.zeros(shape, dtype))
        n_params = len(in_names)
        n_outs = len(out_avals)
        all_in_names = list(in_names) + list(out_names)
        if partition_name is not None:
            all_in_names.append(partition_name)
        self.in_names = in_names
        self.out_names = out_names
        self.out_avals = out_avals
        self.zero_outs = zero_outs
        donate = tuple(range(n_params, n_params + n_outs))

        def _body(*args):
            operands = list(args)
            if partition_name is not None:
                operands.append(partition_id_tensor())
            outs = _bass_exec_p.bind(
                *operands,
                out_avals=tuple(out_avals),
                in_names=tuple(all_in_names),
                out_names=tuple(out_names),
                lowering_input_output_aliases=(),
                sim_require_finite=True,
                sim_require_nnan=True,
                nc=nc,
            )
            return tuple(outs)

        devices = jax.devices()[:n_cores]
        assert len(devices) == n_cores, (
            f"need {n_cores} neuron devices, found {len(jax.devices())}"
        )
        mesh = Mesh(np.asarray(devices), ("core",))
        in_specs = (PartitionSpec("core"),) * (n_params + n_outs)
        out_specs = (PartitionSpec("core"),) * n_outs
        self.fn = jax.jit(
            shard_map(_body, mesh=mesh, in_specs=in_specs,
                      out_specs=out_specs, check_rep=False),
            donate_argnums=donate, keep_unused=True,
        )

    def __call__(self, in_maps):
        n = self.n_cores
        cat = [
            np.concatenate([np.asarray(in_maps[c][nm]) for c in range(n)],
                           axis=0)
            for nm in self.in_names
        ]
        zeros = [
            np.zeros((n * z.shape[0], *z.shape[1:]), z.dtype)
            for z in self.zero_outs
        ]
        out_arrs = self.fn(*cat, *zeros)
        self.jax.block_until_ready(out_arrs)
        return [
            {
                nm: np.asarray(out_arrs[i]).reshape(
                    n, *self.out_avals[i].shape)[c]
                for i, nm in enumerate(self.out_names)
            }
            for c in range(n)
        ]


_RUNNERS = {}


def _get_runner(name):
    if name not in _RUNNERS:
        nc = _build_attn() if name == "attn" else _build_mlp()
        _RUNNERS[name] = _CompiledSpmd(nc, N_CORES)
    return _RUNNERS[name]


# ---------------------------------------------------------------------------
# Host-side sharding / weight folding
# ---------------------------------------------------------------------------

def _prep_attn_inmaps(x, w_qkv, b_qkv, ln1_g, ln1_b):
    maps = []
    for core in range(N_CORES):
        b = core // 4
        hp = core % 4
        cols = np.concatenate([
            np.arange(hp * 128, (hp + 1) * 128),
            np.arange(C + hp * 128, C + (hp + 1) * 128),
            np.arange(2 * C + hp * 128, 2 * C + (hp + 1) * 128),
        ])
        wslice = w_qkv[:, cols]
        beff = b_qkv[cols] + ln1_b @ wslice
        weff = ln1_g[:, None] * wslice
        maps.append({
            "xb": np.ascontiguousarray(x[b], dtype=np.float32),
            "wqkv": np.ascontiguousarray(
                weff.reshape(4, 128, 384), dtype=np.float32),
            "bqkv": np.ascontiguousarray(
                beff.reshape(3, 128), dtype=np.float32),
        })
    return maps


def _prep_mlp_inmaps(x, yT_by_batch, w_attn_proj, b_attn_proj,
                     w_fc, b_fc, w_mlp_proj, b_mlp_proj, ln2_g, ln2_b):
    wfc_eff = (ln2_g[:, None] * w_fc).astype(np.float32)
    bfc_eff = (b_fc + ln2_b @ w_fc).astype(np.float32)
    wap = np.ascontiguousarray(w_attn_proj.reshape(4, 128, C),
                               dtype=np.float32)
    wfc = np.ascontiguousarray(wfc_eff.reshape(4, 128, 4 * C))
    bfc = np.ascontiguousarray(bfc_eff.reshape(16, 128))
    import ml_dtypes
    wmp = np.ascontiguousarray(w_mlp_proj.reshape(16, 128, C)
                               .astype(ml_dtypes.bfloat16))
    bmp = np.ascontiguousarray(b_mlp_proj.reshape(4, 128), dtype=np.float32)
    xb_eff = (x + b_attn_proj[None, None, :]).astype(np.float32)
    maps = []
    for core in range(N_CORES):
        t0 = core * 1024
        b = t0 // T
        tl = t0 % T
        maps.append({
            "yTc": np.ascontiguousarray(yT_by_batch[b][:, tl:tl + 1024]),
            "xc": np.ascontiguousarray(xb_eff[b, tl:tl + 1024]),
            "wap": wap, "wfc": wfc, "bfc": bfc,
            "wmp": wmp, "bmp": bmp,
        })
    return maps


# ---------------------------------------------------------------------------
# Public entry point
# ---------------------------------------------------------------------------

def kernel(x, w_qkv, b_qkv, w_attn_proj, b_attn_proj, w_fc, b_fc,
           w_mlp_proj, b_mlp_proj, ln1_g, ln1_b, ln2_g, ln2_b):
    x = np.asarray(x, dtype=np.float32)
    w_qkv = np.asarray(w_qkv, dtype=np.float32)
    b_qkv = np.asarray(b_qkv, dtype=np.float32)
    w_attn_proj = np.asarray(w_attn_proj, dtype=np.float32)
    b_attn_proj = np.asarray(b_attn_proj, dtype=np.float32)
    w_fc = np.asarray(w_fc, dtype=np.float32)
    b_fc = np.asarray(b_fc, dtype=np.float32)
    w_mlp_proj = np.asarray(w_mlp_proj, dtype=np.float32)
    b_mlp_proj = np.asarray(b_mlp_proj, dtype=np.float32)
    ln1_g = np.asarray(ln1_g, dtype=np.float32)
    ln1_b = np.asarray(ln1_b, dtype=np.float32)
    ln2_g = np.asarray(ln2_g, dtype=np.float32)
    ln2_b = np.asarray(ln2_b, dtype=np.float32)

    am = _prep_attn_inmaps(x, w_qkv, b_qkv, ln1_g, ln1_b)
    outs_a = _get_runner("attn")(am)
    yT_by_batch = [
        np.concatenate([outs_a[b * 4 + i]["yT"] for i in range(4)], axis=0)
        for b in range(2)
    ]
    mm = _prep_mlp_inmaps(x, yT_by_batch, w_attn_proj, b_attn_proj, w_fc,
                          b_fc, w_mlp_proj, b_mlp_proj, ln2_g, ln2_b)
    outs_b = _get_runner("mlp")(mm)
    out = np.empty((2, T, C), np.float32)
    for core in range(N_CORES):
        t0 = core * 1024
        out[t0 // T, t0 % T: t0 % T + 1024] = outs_b[core]["outc"]
    return out


def timing_launches(inputs):
    """For test.py: [(name, build_fn(repeat=), in_maps)] per SPMD launch."""
    ins = {k: np.asarray(v, dtype=np.float32) for k, v in inputs.items()}
    am = _prep_attn_inmaps(ins["x"], ins["w_qkv"], ins["b_qkv"],
                           ins["ln1_g"], ins["ln1_b"])
    outs_a = _get_runner("attn")(am)
    yT_by_batch = [
        np.concatenate([outs_a[b * 4 + i]["yT"] for i in range(4)], axis=0)
        for b in range(2)
    ]
    mm = _prep_mlp_inmaps(ins["x"], yT_by_batch, ins["w_attn_proj"],
                          ins["b_attn_proj"], ins["w_fc"], ins["b_fc"],
                          ins["w_mlp_proj"], ins["b_mlp_proj"],
                          ins["ln2_g"], ins["ln2_b"])
    return [("attn", _build_attn, am), ("mlp", _build_mlp, mm)]
